# revision 1
# baseline (speedup 1.0000x reference)
"""GATv2 message-passing kernel for 8 Trainium2 NeuronCores (Bass/Tile), v2.

Strategy (edge parallelism over receiver-sorted edges), changes vs v1:
  * bf16 on every matmul/gather/streaming path (PE runs 1 cycle/row instead
    of 4; streaming DMA halves).  fp32 kept for PSUM accumulation and the
    logits; the mish chain runs bf16 on the vector engine where possible.
  * The receiver one-hot matrices (both orientations) are precomputed on the
    host in fp8e4 and streamed from HBM with ONE DMA per position (GTh:
    [128 rloc, edge] for the h_recv scatter; Gtf: [edge, rloc] per group for
    the segment-sum matmuls).  This removes the ones-broadcast matmul and
    all DVE is_equal builds, and keeps the HWDGE queue short.
  * es transposes into the attention PSUM run as regular matmuls against a
    bf16 identity (is_transpose would force a bf16 PSUM dtype).
  * Phase A (h = nf @ W) is bf16 with a single [128, 1024] PSUM tile per
    sweep, one interleaved nfT DMA, one ACT Copy, one DMA out.
  * CHUNK_G=8 (1024-edge chunks) halves per-instruction overheads.
"""

import ml_dtypes
import numpy as np

import concourse.bass as bass
import concourse.bacc as bacc
import concourse.tile as tile
from concourse import mybir
from concourse.bass_utils import run_bass_kernel_spmd
from concourse.tile_rust import add_dep_helper

F32 = mybir.dt.float32
BF16 = mybir.dt.bfloat16
FP8 = mybir.dt.float8e4
I32 = mybir.dt.int32
AF = mybir.ActivationFunctionType
OP = mybir.AluOpType
BF = ml_dtypes.bfloat16
F8 = ml_dtypes.float8_e4m3fn

# The act-table-load pass assigns each activation the FIRST table set whose
# function list contains it.  By default Exp->exp_and_others and
# Ln->natural_log, which puts this kernel's exp/ln chain in different sets
# and forces a 1.28us table reload between almost every ACT op.  Emptying
# those two sets (ids preserved) makes Exp and Ln co-resolve to
# natural_log_exp_and_others and Tanh to sigmoid_and_others: 2 reloads per
# receiver block instead of ~6.
import contextlib
import concourse.hw_specs as _hw_specs
import concourse.bacc as _bacc_mod


@contextlib.contextmanager
def _act_table_patch():
    """Scoped to our own finalize() so the process-global activation-table
    view stays pristine for anything else compiled in this process."""
    orig = _hw_specs.get_activation_tables

    def _patched(arch):
        t = dict(orig(arch))
        for k in ("exp_and_others", "natural_log"):
            if k in t:
                t[k] = set()
        return t

    _hw_specs.get_activation_tables = _patched
    _bacc_mod.get_activation_tables = _patched
    try:
        yield
    finally:
        _hw_specs.get_activation_tables = orig
        _bacc_mod.get_activation_tables = orig

N_NODES = 50000
N_EDGES = 800000
IN_DIM = 256
EDGE_DIM = 64
EMBED = 128
HEADS = 8
HEAD_DIM = EMBED // HEADS
P = 128
NCORES = 8
CHUNK_G = 8  # groups (of 128 edges) per processing chunk
PAD_RLOC = 200.0  # sentinel receiver-local id for padding edges (> 127)


# ---------------------------------------------------------------- host plan

def _plan(receivers, senders, n_nodes, ncores):
    """Sort edges by receiver, then by sender within each 128-node receiver
    block (so gathers use monotone addresses and fit int16 index windows);
    deal blocks to cores balanced by edge count; pad every (core, position)
    to a common group count; pick per-(position, chunk) gather base offsets
    shared by all cores."""
    order = np.argsort(receivers, kind="stable").astype(np.int64)
    r_s = receivers[order].astype(np.int64)
    nb = -(-n_nodes // P)
    npos = -(-nb // ncores)
    nb_pad = npos * ncores
    n_pad = nb_pad * P
    cnt = np.bincount(r_s // P, minlength=nb_pad).astype(np.int64)
    estart = np.zeros(nb_pad, np.int64)
    estart[1:] = np.cumsum(cnt)[:-1]
    # sender-sort within each receiver block
    for b in range(nb_pad):
        e0, c = int(estart[b]), int(cnt[b])
        if c > 1:
            seg = order[e0:e0 + c]
            order[e0:e0 + c] = seg[np.argsort(senders[seg], kind="stable")]
    r_s = receivers[order].astype(np.int64)
    gcnt = np.maximum(-(-cnt // P), 1)
    deal = np.argsort(-gcnt, kind="stable")
    blocks = deal.reshape(npos, ncores)  # blocks[pos, core] -> block id
    gpos = gcnt[blocks].max(axis=1)      # groups per position (same all cores)
    goff = np.zeros(npos, np.int64)
    goff[1:] = np.cumsum(gpos)[:-1]
    # per-(position, chunk) gather windows, uniform across cores.  A gather
    # descriptor is (group offset within position, n groups, base row); when
    # a full chunk's sender window overflows int16, split it in halves.
    def _window(pos, glo, ghi):
        lo, hi = np.iinfo(np.int64).max, 0
        for core in range(ncores):
            b = int(blocks[pos, core])
            e0, c = int(estart[b]), int(cnt[b])
            sc = senders[order[e0:e0 + c]].astype(np.int64)
            part = sc[glo * P:min(ghi * P, c)]
            if part.size:
                lo = min(lo, int(part.min()))
                hi = max(hi, int(part.max()))
        if lo > hi:
            return 0, 0
        return lo, hi

    gdesc = []
    for pos in range(npos):
        gp = int(gpos[pos])
        descs = []
        for ch in range(-(-gp // CHUNK_G)):
            g_lo = ch * CHUNK_G
            g_hi = min(g_lo + CHUNK_G, gp)
            lo, hi = _window(pos, g_lo, g_hi)
            if hi - lo < 32768:
                descs.append((g_lo, g_hi - g_lo, lo))
            else:
                mid = g_lo + (g_hi - g_lo + 1) // 2
                for a_, b_ in ((g_lo, mid), (mid, g_hi)):
                    lo, hi = _window(pos, a_, b_)
                    assert hi - lo < 32768, \
                        f"gather window overflow at pos {pos}: {hi - lo}"
                    descs.append((a_, b_ - a_, lo))
        gdesc.append(descs)
    return dict(order=order, r_s=r_s, cnt=cnt, estart=estart, blocks=blocks,
                gpos=gpos, goff=goff, gtot=int(gpos.sum()),
                ecap=int(gpos.sum()) * P, npos=npos, nb_pad=nb_pad,
                gdesc=gdesc, n_pad=n_pad)


def _host_inputs(plan, node_features, edge_features, W_kernel, W_bias,
                 We_kernel, We_bias, a, senders):
    """Build the per-core input maps (all numpy, no math beyond transposes)."""
    npos, gtot, ecap = plan["npos"], plan["gtot"], plan["ecap"]
    n_pad = plan["nb_pad"] * P
    n_nodes, in_dim = node_features.shape
    heads, head_dim = a.shape
    embed = heads * head_dim
    edge_dim = edge_features.shape[1]
    HW_ = 8 * P  # phase A nodes per sweep

    # interleaved nfT: per sweep of 1024 nodes, k-rows 0:128 then 128:256
    nfT = np.zeros((in_dim, n_pad), np.float32)
    nfT[:, :n_nodes] = node_features.T
    nfT2 = np.zeros((P, 2 * n_pad), BF)
    for s in range(n_pad // HW_):
        nfT2[:, 2 * s * HW_:2 * s * HW_ + HW_] = nfT[0:P, s * HW_:(s + 1) * HW_]
        nfT2[:, 2 * s * HW_ + HW_:2 * (s + 1) * HW_] = \
            nfT[P:2 * P, s * HW_:(s + 1) * HW_]
    We_aug = np.concatenate(
        [We_kernel, (We_bias + 2.0 * W_bias)[None, :]], axis=0
    ).astype(BF)
    A_blk = np.zeros((embed, heads), np.float32)
    for h in range(heads):
        A_blk[h * head_dim:(h + 1) * head_dim, h] = a[h]
    Wb_rep = np.tile(W_bias[None, :], (P, 1)).astype(np.float32)
    identity = np.eye(P, dtype=BF)

    efT_all = np.ascontiguousarray(edge_features[plan["order"]].T).astype(BF)
    s_sorted = senders[plan["order"]].astype(np.int32)
    rloc_all = (plan["r_s"] - (plan["r_s"] // P) * P).astype(np.int64)

    qrange = np.arange(P, dtype=np.int64)
    shared = {
        "nfT2": nfT2, "W": W_kernel.astype(BF), "We_aug": We_aug,
        "A_blk": A_blk.astype(BF), "Wb_rep": Wb_rep, "identity": identity,
    }
    in_maps = []
    for core in range(NCORES):
        senders16 = np.zeros((P, gtot * 8), np.int16)
        efTa = np.zeros((edge_dim + 1, ecap), BF)
        efTa[edge_dim, :] = 1.0
        rl_flat = np.full(ecap, int(PAD_RLOC), np.int64)
        blocknodes = np.zeros((P, npos), np.int32)
        for pos in range(npos):
            b = int(plan["blocks"][pos, core])
            g0 = int(plan["goff"][pos])
            gp = int(plan["gpos"][pos])
            c = int(plan["cnt"][b])
            e0 = int(plan["estart"][b])
            blocknodes[:, pos] = b * P + np.arange(P)
            col0 = g0 * P
            efTa[:edge_dim, col0:col0 + c] = efT_all[:, e0:e0 + c]
            rl_flat[col0:col0 + c] = rloc_all[e0:e0 + c]
            for (g_lo, ng, base) in plan["gdesc"][pos]:
                s_ch = ng * P
                tmp_s = np.full(s_ch, base, np.int64)  # pads -> row `base`
                r0 = g_lo * P
                nreal = min(max(c - r0, 0), s_ch)
                tmp_s[:nreal] = s_sorted[e0 + r0:e0 + r0 + nreal]
                rel = (tmp_s - base).astype(np.int16)
                blk16 = np.tile(rel.reshape(s_ch // 16, 16).T, (8, 1))
                cb = (g0 * P + r0) // 16
                senders16[:, cb:cb + s_ch // 16] = blk16
        # receiver one-hots, both orientations, fp8 (pads -> all-zero),
        # packed per position as [GTh_pos | Gtf_pos] for one DMA each
        GTh = (rl_flat[None, :] == qrange[:, None])
        rl_g = rl_flat.reshape(gtot, P)  # [g, p] -> rloc of edge g*128+p
        Gtf = (rl_g[:, :, None] == qrange[None, None, :])  # [g, p, q]
        GG = np.zeros((P, 2 * ecap), F8)
        for pos in range(npos):
            g0 = int(plan["goff"][pos])
            gp = int(plan["gpos"][pos])
            for cc in range(-(-gp // CHUNK_G)):
                gc_ = min(CHUNK_G, gp - cc * CHUNK_G)
                sc_ = gc_ * P
                gl = g0 + cc * CHUNK_G
                c0 = 2 * (g0 * P + cc * CHUNK_G * P)
                GG[:, c0:c0 + sc_] = \
                    GTh[:, gl * P:gl * P + sc_].astype(F8)
                GG[:, c0 + sc_:c0 + 2 * sc_] = np.ascontiguousarray(
                    Gtf[gl:gl + gc_].transpose(1, 0, 2).reshape(P, sc_)
                ).astype(F8)
        m = dict(shared)
        m.update({"senders16": senders16, "efTa": efTa,
                  "GG": GG, "blocknodes": blocknodes})
        in_maps.append(m)
    return in_maps


# ---------------------------------------------------------------- bass build

def _build(plan, n_pad, in_dim, edge_dim, embed, heads, debug=False,
           repeat=1, parts="full"):
    head_dim = embed // heads
    npos, gtot, ecap = plan["npos"], plan["gtot"], plan["ecap"]
    gpos, goff = plan["gpos"], plan["goff"]
    gpmax = int(gpos.max())
    UW = embed + heads  # U columns: [weighted sum | denom]

    nc = bacc.Bacc("TRN2", num_swdge_queues=4,
                   dynamic_dma_scratch_size=65536)
    t_nfT2 = nc.dram_tensor("nfT2", [P, 2 * n_pad], BF16,
                            kind="ExternalInput")
    t_W = nc.dram_tensor("W", [in_dim, embed], BF16, kind="ExternalInput")
    t_We = nc.dram_tensor("We_aug", [edge_dim + 1, embed], BF16,
                          kind="ExternalInput")
    t_A = nc.dram_tensor("A_blk", [embed, heads], BF16, kind="ExternalInput")
    t_Wb = nc.dram_tensor("Wb_rep", [P, embed], F32, kind="ExternalInput")
    t_id = nc.dram_tensor("identity", [P, P], BF16, kind="ExternalInput")
    t_s16 = nc.dram_tensor("senders16", [P, gtot * 8], mybir.dt.int16,
                           kind="ExternalInput")
    t_efT = nc.dram_tensor("efTa", [edge_dim + 1, ecap], BF16,
                           kind="ExternalInput")
    t_GG = nc.dram_tensor("GG", [P, 2 * ecap], FP8, kind="ExternalInput")
    t_bn = nc.dram_tensor("blocknodes", [P, npos], I32, kind="ExternalInput")
    t_out = nc.dram_tensor("out", [npos * P, embed], F32,
                           kind="ExternalOutput")
    t_h = nc.dram_tensor("h_scratch", [n_pad, embed], BF16, kind="Internal")

    with tile.TileContext(nc) as tc:
        with tc.tile_pool(name="const", bufs=1) as cp:
            def cload(t, shape):
                s = cp.tile(shape, t.dtype, tag=f"c_{t.name}")
                nc.sync.dma_start(out=s[:], in_=t[:])
                return s

            W0 = cp.tile([P, embed], BF16)
            nc.sync.dma_start(out=W0[:], in_=t_W[0:P, :])
            W1 = cp.tile([P, embed], BF16)
            nc.sync.dma_start(out=W1[:], in_=t_W[P:2 * P, :])
            We = cload(t_We, [edge_dim + 1, embed])
            Ab = cload(t_A, [embed, heads])
            Wb = cload(t_Wb, [P, embed])
            idn = cload(t_id, [P, P])
            s16 = cload(t_s16, [P, gtot * 8])
            bn = cload(t_bn, [P, npos])

            # ---------------- phase A: h = nf @ W (no bias) ----------------
            for _rep in range(repeat):
              with tc.tile_pool(name=f"ha{_rep}", bufs=4) as hap, \
                      tc.tile_pool(name=f"haps{_rep}", bufs=3, space="PSUM") as hpp:
                  HW_ = 8 * P  # nodes per sweep
                  for nt in range(n_pad // HW_):
                      na = hap.tile([P, 2 * HW_], BF16, tag="nfT0")
                      nc.sync.dma_start(
                          out=na[:],
                          in_=t_nfT2[:, 2 * nt * HW_:2 * (nt + 1) * HW_])
                      hstage = hap.tile([P, HW_], BF16, tag="hstage")
                      hp = hpp.tile([P, HW_], F32, tag="hps")
                      for t in range(HW_ // P):
                          nc.tensor.matmul(hp[:, t * P:(t + 1) * P],
                                           lhsT=na[:, t * P:(t + 1) * P],
                                           rhs=W0[:], start=True, stop=False)
                          nc.tensor.matmul(hp[:, t * P:(t + 1) * P],
                                           lhsT=na[:, HW_ + t * P:
                                                    HW_ + (t + 1) * P],
                                           rhs=W1[:], start=False, stop=True)
                      nc.scalar.activation(out=hstage[:], in_=hp[:],
                                           func=AF.Copy)
                      out_view = bass.AP(
                          t_h[:].tensor, nt * HW_ * embed,
                          [[embed, P], [P * embed, HW_ // P], [1, embed]])
                      nc.scalar.dma_start(out=out_view, in_=hstage[:])

              tc.strict_bb_all_engine_barrier()
              if parts == "a":
                  continue

              # ---------------- phase B: edge processing ---------------------
              # Positions are processed in interleaved groups of G_ILV so the
              # ACT table set only switches twice per GROUP (stage 1 of all
              # members uses {exp, ln}; stage 2 + the block exps use
              # {tanh, exp}).
              G_ILV = 3
              with tc.tile_pool(name=f"eb{_rep}", bufs=6) as ep, \
                      tc.tile_pool(name=f"ebsm{_rep}", bufs=3) as esm, \
                      tc.tile_pool(name=f"ebp{_rep}", bufs=2, space="PSUM") as pp, \
                      tc.tile_pool(name=f"ups{_rep}", bufs=2, space="PSUM") as up:
                  prev_exp_ins = None
                  nd2_tiles = []
                  qi = 0  # gather queue rotation (1..3; 0 = indirect DMAs)
                  for p0 in range(0, npos, G_ILV):
                      grp = list(range(p0, min(p0 + G_ILV, npos)))
                      # issue output stores lagged by 2 groups: the data is
                      # long since ready, so the SP queue never blocks on it
                      while len(nd2_tiles) > 2 * G_ILV:
                          pos_, t_ = nd2_tiles.pop(0)
                          nc.sync.dma_start(
                              out=t_out[pos_ * P:(pos_ + 1) * P, :],
                              in_=t_[:])
                      st = {}
                      UpsG = up.tile([P, G_ILV * UW], F32, tag="U", bufs=1)
                      # --- stage 1: pre-activation x and softplus(x) ---------
                      # (ACT stays on the {exp, ln} table set here)
                      last_sp_ins = None
                      for pos in grp:
                          g_here = int(gpos[pos])
                          g0 = int(goff[pos])
                          Hb = ep.tile([P, embed], BF16, tag="Hb")
                          nc.gpsimd.indirect_dma_start(
                              out=Hb[:], out_offset=None, in_=t_h[:],
                              in_offset=bass.IndirectOffsetOnAxis(
                                  ap=bn[:, pos:pos + 1], axis=0))
                          k_ = pos - p0
                          Ups = UpsG[:, k_ * UW:(k_ + 1) * UW]
                          lgb = up.tile([P, ((gpmax * heads + 127) // 128)
                                         * 128], F32, tag="lgb", bufs=1)
                          nchunks = -(-g_here // CHUNK_G)
                          d = dict(g_here=g_here, g0=g0, ggc=[], Ups=Ups,
                                   lgb=lgb, nchunks=nchunks, es=[], sp=[],
                                   xc=[])
                          st[pos] = d
                          for c in range(nchunks):
                              gc = min(CHUNK_G, g_here - c * CHUNK_G)
                              s = gc * P
                              co = c * CHUNK_G * P
                              es = ep.tile([P, CHUNK_G * P], BF16, tag="es",
                                           bufs=11)
                              d["es"].append(es)
                              ggc = ep.tile([P, 2 * CHUNK_G * P], FP8,
                                            tag="ggc", bufs=10)
                              d["ggc"].append(ggc)
                              nc.sync.dma_start(
                                  out=ggc[:, :2 * s],
                                  in_=t_GG[:, 2 * (g0 * P + co):
                                           2 * (g0 * P + co) + 2 * s])
                              efc = ep.tile([edge_dim + 1, CHUNK_G * P], BF16,
                                            tag="efc", bufs=6)
                              nc.sync.dma_start(
                                  out=efc[:, :s],
                                  in_=t_efT[:, g0 * P + co:g0 * P + co + s])
                              for (g_lo, ng, base) in plan["gdesc"][pos]:
                                  if not (c * CHUNK_G <= g_lo
                                          < c * CHUNK_G + gc):
                                      continue
                                  rows = min(n_pad - base, 32768)
                                  cb = g0 * 8 + g_lo * 8
                                  o0 = (g_lo - c * CHUNK_G) * P
                                  sg = ng * P
                                  nc.gpsimd.dma_gather(
                                      out_ap=es[:, o0:o0 + sg].rearrange(
                                          "p (j e) -> p j e", e=embed),
                                      in_ap=t_h[base:base + rows, :],
                                      idxs_ap=s16[:, cb:cb + sg // 16],
                                      num_idxs=sg, num_idxs_reg=sg,
                                      elem_size=embed,
                                      queue_num=1 + qi % 3)
                                  qi += 1
                              if parts == "ag":
                                  continue
                              at = pp.tile([P, CHUNK_G * P], F32, tag="attnT",
                                           bufs=3)
                              for o_ in range(0, s, 512):
                                  w_ = min(512, s - o_)
                                  nc.tensor.matmul(at[:, o_:o_ + w_],
                                                   lhsT=We[:],
                                                   rhs=efc[:, o_:o_ + w_],
                                                   start=True, stop=False)
                                  nc.tensor.matmul(at[:, o_:o_ + w_],
                                                   lhsT=Hb[:],
                                                   rhs=ggc[:, o_:o_ + w_],
                                                   start=False, stop=False)
                              for j in range(gc):
                                  # es_j^T via regular matmul against identity
                                  # (is_transpose would force bf16 PSUM out)
                                  nc.tensor.matmul(
                                      at[:, j * P:(j + 1) * P],
                                      lhsT=es[:, j * P:(j + 1) * P],
                                      rhs=idn[:],
                                      start=False, stop=(j == gc - 1))
                              # mish(x) = x * tanh(ln(1 + exp(x))) — composed
                              # from table-mapped functions.  xc copies x out
                              # of PSUM.
                              xc = ep.tile([P, CHUNK_G * P], BF16, tag="xc",
                                           bufs=11)
                              d["xc"].append(xc)
                              nc.vector.tensor_copy(out=xc[:, :s],
                                                    in_=at[:, :s])
                              vv = ep.tile([P, CHUNK_G * P], F32, tag="vv", bufs=4)
                              v_ins = nc.scalar.activation(out=vv[:, :s],
                                                           in_=at[:, :s],
                                                           func=AF.Exp)
                              if prev_exp_ins is not None:
                                  # keep ACT's {exp,ln} ops grouped after the
                                  # previous group's {tanh,exp} ops so
                                  # lower_act emits 2 table loads per group
                                  add_dep_helper(v_ins.ins, prev_exp_ins,
                                                 sync=False,
                                                 reason="act table grouping")
                                  prev_exp_ins = None
                              sp = ep.tile([P, CHUNK_G * P], BF16, tag="sp",
                                           bufs=10)
                              d["sp"].append(sp)
                              sp_ins = nc.scalar.activation(out=sp[:, :s],
                                                            in_=vv[:, :s],
                                                            func=AF.Ln,
                                                            bias=1.0)
                              last_sp_ins = sp_ins
                      if parts == "ag":
                          continue
                      # --- stage 2: tanh, mish, logits -----------------------
                      # (ACT switches to the {tanh, exp} table set)
                      first_tanh = True
                      for pos in grp:
                          d = st[pos]
                          for c in range(d["nchunks"]):
                              gc = min(CHUNK_G, d["g_here"] - c * CHUNK_G)
                              s = gc * P
                              mi = ep.tile([P, CHUNK_G * P], BF16, tag="mish", bufs=4)
                              t_ins = nc.scalar.activation(
                                  out=mi[:, :s], in_=d["sp"][c][:, :s],
                                  func=AF.Tanh)
                              if first_tanh:
                                  add_dep_helper(t_ins.ins, last_sp_ins.ins,
                                                 sync=False,
                                                 reason="act table grouping")
                                  first_tanh = False
                              nc.vector.tensor_tensor(out=mi[:, :s],
                                                      in0=d["xc"][c][:, :s],
                                                      in1=mi[:, :s],
                                                      op=OP.mult)
                              for j in range(gc):
                                  nc.tensor.matmul(
                                      d["lgb"][:, (c * CHUNK_G + j) * heads:
                                               (c * CHUNK_G + j + 1) * heads],
                                      lhsT=mi[:, j * P:(j + 1) * P],
                                      rhs=Ab[:], start=True, stop=True)
                          # copy logits to SBUF so the lgb PSUM bank frees
                          # before the next position; the block exps run
                          # grouped after all tanhs (one {exp,ln} table load
                          # per group, adjacent to the next group's stage 1)
                          lgbs = ep.tile([P, gpmax * heads], F32, tag="lgbs",
                                         bufs=3)
                          d["lgbs"] = lgbs
                          nc.vector.tensor_copy(
                              out=lgbs[:, :d["g_here"] * heads],
                              in_=d["lgb"][:, :d["g_here"] * heads])
                      for pos in grp:
                          d = st[pos]
                          exb = esm.tile([P, gpmax * heads], BF16, tag="exb")
                          d["exb"] = exb
                          exp_ins = nc.scalar.activation(
                              out=exb[:, :d["g_here"] * heads],
                              in_=d["lgbs"][:, :d["g_here"] * heads],
                              func=AF.Exp)
                          prev_exp_ins = exp_ins.ins
                      # --- stage 3: weighted scatter-accumulate --------------
                      for pos in grp:
                          d = st[pos]
                          g_here = d["g_here"]
                          exb = d["exb"]
                          Ups = d["Ups"]
                          nchunks = d["nchunks"]
                          for c in range(nchunks):
                              gc = min(CHUNK_G, g_here - c * CHUNK_G)
                              s = gc * P
                              es = d["es"][c]
                              rb = ep.tile([P, CHUNK_G * UW], BF16,
                                           tag="rhsb", bufs=4)
                              rb3 = rb[:].rearrange("p (j c) -> p j c",
                                                    j=CHUNK_G)
                              ex_view = rb3[:, :gc, embed:UW]
                              exb_view = exb[:, c * CHUNK_G * heads:
                                             (c * CHUNK_G + gc) *
                                             heads].rearrange(
                                  "p (j h) -> p j h", j=gc)
                              nc.vector.tensor_copy(out=ex_view, in_=exb_view)
                              m_view = rb3[:, :gc, 0:embed].rearrange(
                                  "p j (h w) -> p j h w", w=head_dim)
                              es_view = es[:, :s].rearrange(
                                  "p (j h w) -> p j h w", j=gc, w=head_dim)
                              ex_b = exb_view.to_broadcast(
                                  [P, gc, heads, head_dim])
                              nc.vector.tensor_tensor(out=m_view, in0=es_view,
                                                      in1=ex_b, op=OP.mult)
                              for j in range(gc):
                                  nc.tensor.matmul(
                                      Ups[:],
                                      lhsT=d["ggc"][c][
                                          :, s + j * P:s + (j + 1) * P],
                                      rhs=rb[:, j * UW:(j + 1) * UW],
                                      start=(c == 0 and j == 0),
                                      stop=(c == nchunks - 1 and j == gc - 1))
                          # -- block epilogue: out = U / max(denom, eps) + Wb --
                          dn = ep.tile([P, heads], F32, tag="dn")
                          nc.vector.tensor_scalar(out=dn[:],
                                                  in0=Ups[:, embed:UW],
                                                  scalar1=1e-30, scalar2=None,
                                                  op0=OP.max)
                          rc = ep.tile([P, heads], F32, tag="rc")
                          nc.vector.reciprocal(rc[:], dn[:])
                          nd = ep.tile([P, embed], F32, tag="nodes")
                          ndv = nd[:].rearrange("p (h w) -> p h w", w=head_dim)
                          uv = Ups[:, 0:embed].rearrange("p (h w) -> p h w",
                                                         w=head_dim)
                          rcb = rc[:].to_broadcast([P, heads, head_dim])
                          nc.vector.tensor_tensor(out=ndv, in0=uv, in1=rcb,
                                                  op=OP.mult)
                          nd2 = ep.tile([P, embed], F32, tag="nodes2",
                                        bufs=8)
                          nc.vector.tensor_tensor(out=nd2[:], in0=nd[:],
                                                  in1=Wb[:], op=OP.add)
                          nd2_tiles.append((pos, nd2))
                  # deferred output stores: keep the SP queue free of
                  # pipeline-dependent stores during the main loop
                  for pos_, t_ in nd2_tiles:
                      nc.sync.dma_start(
                          out=t_out[pos_ * P:(pos_ + 1) * P, :], in_=t_[:])
    with _act_table_patch():
        nc.finalize()
    return nc


# ---------------------------------------------------------------- entry

def _run(node_features, edge_features, W_kernel, W_bias, We_kernel, We_bias,
         a, senders, receivers, trace=False):
    n_nodes, in_dim = node_features.shape
    heads, head_dim = a.shape
    embed = heads * head_dim
    edge_dim = edge_features.shape[1]
    plan = _plan(receivers, senders, n_nodes, NCORES)
    n_pad = plan["nb_pad"] * P
    in_maps = _host_inputs(plan, node_features, edge_features, W_kernel,
                           W_bias, We_kernel, We_bias, a, senders)
    nc = _build(plan, n_pad, in_dim, edge_dim, embed, heads)
    res = run_bass_kernel_spmd(nc, in_maps, core_ids=list(range(NCORES)),
                               trace=trace)
    # reassemble: core outputs are [npos*P, embed]; position rows -> blocks
    out = np.zeros((n_pad, embed), np.float32)
    for core in range(NCORES):
        o = res.results[core]["out"]
        for pos in range(plan["npos"]):
            b = int(plan["blocks"][pos, core])
            out[b * P:(b + 1) * P] = o[pos * P:(pos + 1) * P]
    out = out[:n_nodes]
    # nodes with no incoming edges: reference segment_sum gives exactly 0
    deg = np.bincount(receivers.astype(np.int64), minlength=n_nodes)
    if (deg == 0).any():
        out[deg == 0] = 0.0
    return out, res


def kernel(node_features, edge_features, W_kernel, W_bias, We_kernel,
           We_bias, a, senders, receivers):
    node_features = np.asarray(node_features, np.float32)
    edge_features = np.asarray(edge_features, np.float32)
    W_kernel = np.asarray(W_kernel, np.float32)
    W_bias = np.asarray(W_bias, np.float32)
    We_kernel = np.asarray(We_kernel, np.float32)
    We_bias = np.asarray(We_bias, np.float32)
    a = np.asarray(a, np.float32)
    senders = np.asarray(senders, np.int32)
    receivers = np.asarray(receivers, np.int32)
    out, _ = _run(node_features, edge_features, W_kernel, W_bias, We_kernel,
                  We_bias, a, senders, receivers)
    return out

